# revision 24
# baseline (speedup 1.0000x reference)
"""Multi-head attention (B=2, D=2048, N=1024, H=16) on 8 TRN2 NeuronCores.

Sharding: batch*heads across cores - core c handles batch c//4, heads
4*(c%4) .. 4*(c%4)+3. No collectives.

The attention phase is ScalarE-bound: exp of 16.8M score elements per
core at 1 elem/cycle/lane @ 1.2GHz is ~143us of ACTIVATE, which the PE
cannot outrun.  The kernel is organized to (a) start the exp stream as
early as possible and (b) hide ALL projection work in PE idle during the
ACT-bound attention phase:

  1. Inputs in fp16 (halves the x DMA; q/k are consumed at fp16 and v at
     bf16 downstream anyway).
  2. Head-PAIR packed blocks: the pair's two heads occupy partitions
     0-63 / 64-127 of the q/k buffer, so their score matmuls run
     CONCURRENTLY as 64x128 row-tiles writing the two bank-halves of one
     [128,1024] PSUM tile; one ACTIVATE exps both heads; PV accumulates
     into per-head [65,512] tiles (ones column = softmax denominator).
     Per-step PE work (~640ns) is well under the 1114ns ACT period.
  3. Boot: every PSUM bank holds a projection accumulator so the
     contractions advance chunk-by-chunk as x streams in: q(pair0,s0|s1)
     + k(pair0,j0-7) + v(heads01, all 16 key tiles, packed [128,128])
     = exactly 8 banks.  PSUM rule: a matmul's start=True clears
     has_written for its whole BANK, so only the first group per bank
     uses start=True; later groups in the bank rely on
     overwrite-where-clear.  The k bias cancels in softmax (j-constant
     shift), so k evacuations are plain ScalarE copies; q bias-adds and
     v copies run on DVE.
  4. Remaining projections are quarter/half-size filler matmul groups
     (<=~450ns) woven one per attention step, ordered so every write
     precedes its first reader in program order with slack.  Each
     block's first two score tiles are emitted in the previous block's
     tail steps so the ACTIVATE stream never pauses at block boundaries.

No softmax max-subtraction: scores are ~N(0, 8^2), exp fits fp32.
Host post-pass: divide by denominator, add the (linearly separable) v
bias, transpose + reshape into the reference's raw (B,H,D,p)->(B,D,N).
"""
import sys

sys.path.insert(0, "/opt/trn_rl_repo")

import numpy as np
import ml_dtypes
import concourse.bacc as bacc
import concourse.mybir as mybir
from concourse import tile
from concourse.bass_utils import run_bass_kernel_spmd

B, D, N, H, P = 2, 2048, 1024, 16, 64
NCORES = 8
HPC = 4            # heads per core
KT = 8             # contraction tiles (N / 128)
JT = 16            # j (key) tiles of 128 per head
F32 = mybir.dt.float32
BF16 = mybir.dt.bfloat16
F16 = mybir.dt.float16
EXP = mybir.ActivationFunctionType.Exp

PJ_DT = F16        # projection operands (x, W)
QK_DT = F16        # q/k tiles feeding the scores matmul
PV_DT = BF16       # expS + v_ext feeding the PV matmul

_nc = None


class _PairBlock:
    """One (pair, qc) flash-attention block: heads 2*pair, 2*pair+1,
    query chunk qc (512 wide), 16 flash steps over key tiles."""

    def __init__(self, nc, pair, qc, ps, po, es, obp, qkT, vx, o):
        self.nc, self.pair, self.qc = nc, pair, qc
        self.ps, self.po, self.es, self.obp = ps, po, es, obp
        self.qkT, self.vx, self.o = qkT, vx, o
        self.qoff = pair * D + qc * 512
        self.koff = (2 + pair) * D
        self.sts = {}

    def emit_qk(self, j):
        # both heads concurrently: head A on partitions 0-63 (row tile
        # (0,0)) -> bank half 0, head B on 64-127 ((64,0)) -> half 1
        nc, qkT = self.nc, self.qkT
        st = self.ps.tile([128, 1024], F32, tag="st",
                          name=f"st{self.pair}{self.qc}")
        for hb in range(2):
            nc.tensor.matmul(
                st[:, hb * 512:(hb + 1) * 512],
                qkT[hb * 64:hb * 64 + 64,
                    self.koff + j * 128:self.koff + (j + 1) * 128],
                qkT[hb * 64:hb * 64 + 64, self.qoff:self.qoff + 512],
                start=True, stop=True)
        self.sts[j] = st

    def emit_loop(self, fillers, nxt):
        nc = self.nc
        ots = [self.po.tile([P + 1, 512], F32, tag="po",
                            name=f"ot{self.pair}{self.qc}{hb}")
               for hb in range(2)]
        for j in range(JT):
            et = self.es.tile([128, 1024], PV_DT, tag="et",
                              name=f"et{self.pair}{self.qc}")
            nc.scalar.activation(et[:], self.sts.pop(j)[:], EXP)
            if j + 2 < JT:
                self.emit_qk(j + 2)
            elif nxt is not None:
                nxt.emit_qk(j - 14)
            for hb in range(2):
                # even head: [v|ones] -> denominator in row 64;
                # odd head: [ones|v] (shared ones col) -> denom in row 0
                h = self.pair * 2 + hb
                off = j * 260 + h * 65 - (h % 2)
                nc.tensor.matmul(
                    ots[hb][:],
                    self.vx[:, off:off + 65],
                    et[:, hb * 512:(hb + 1) * 512],
                    start=(j == 0), stop=(j == JT - 1))
            if fillers:
                fillers.pop(0)()
        ob = self.obp.tile([P + 1, 1024], F32, tag="ob")
        orr = self.o.rearrange("h p d -> (h p) d")
        for hb in range(2):
            h = self.pair * 2 + hb
            if nxt is None and hb == 1:
                # last block: ScalarE is idle after the final exp -
                # evacuate the two heads on different engines in parallel
                self.nc.scalar.copy(ob[:, 512:1024], ots[1][:])
            else:
                nc.vector.tensor_copy(ob[:, hb * 512:(hb + 1) * 512],
                                      ots[hb][:])
            nc.sync.dma_start(
                out=orr[h * 65:(h + 1) * 65,
                        self.qc * 512:(self.qc + 1) * 512],
                in_=ob[:, hb * 512:(hb + 1) * 512])


def _build():
    global _nc
    if _nc is not None:
        return _nc
    nc = bacc.Bacc("TRN2", target_bir_lowering=False, debug=False,
                   num_devices=NCORES)
    xt = nc.dram_tensor("xt", [N, D], PJ_DT, kind="ExternalInput").ap()
    # wqk m-major: m in {q lo, q hi, k lo, k hi}, rows m*N + n
    wqk = nc.dram_tensor("wqk", [4 * N, 128], PJ_DT,
                         kind="ExternalInput").ap()
    wv = nc.dram_tensor("wv", [N, HPC * P], PJ_DT, kind="ExternalInput").ap()
    bq = nc.dram_tensor("bq", [128, 2], F32, kind="ExternalInput").ap()
    o = nc.dram_tensor("o", [HPC, P + 1, D], F32, kind="ExternalOutput").ap()

    with tile.TileContext(nc) as tc:
        with tc.tile_pool(name="big", bufs=1) as big, \
             tc.tile_pool(name="es", bufs=10) as es, \
             tc.tile_pool(name="obp", bufs=2) as obp:

            xt_t = big.tile([128, KT * D], PJ_DT, tag="xt")
            wqk_t = big.tile([128, 4 * KT * 128], PJ_DT, tag="wqk")
            wv_t = big.tile([128, KT * 256], PJ_DT, tag="wv")
            bq_t = big.tile([128, 2], F32, tag="bq")
            qkT = big.tile([128, 4 * D], QK_DT, tag="qkT")
            vx = big.tile([128, JT * HPC * 65], PV_DT, tag="vx")
            scr = big.tile([1, 8], F32, tag="scr")

            def wchunk(m, k):
                return wqk_t[:, m * 1024 + k * 128:m * 1024 + (k + 1) * 128]

            def dma_w(m):
                nc.sync.dma_start(
                    out=wqk_t[:, m * 1024:(m + 1) * 1024].rearrange(
                        "p (k c) -> p k c", c=128),
                    in_=wqk[m * N:(m + 1) * N, :].rearrange(
                        "(k p) c -> p k c", p=128))

            # DMA instructions issue serially at ~0.8us each on the Sync
            # queue, so keep the descriptor count low: 3 consolidated
            # weight loads, then the 8 x chunks that pace the boot
            for m in (0, 2):
                dma_w(m)
            nc.sync.dma_start(
                out=wv_t[:].rearrange("p (k c) -> p k c", c=256),
                in_=wv.rearrange("(k p) c -> p k c", p=128))
            for k in range(KT):
                nc.sync.dma_start(out=xt_t[:, k * D:(k + 1) * D],
                                  in_=xt[k * 128:(k + 1) * 128, :])
            nc.sync.dma_start(out=bq_t[:], in_=bq)
            for m in (1, 3):
                dma_w(m)
            # small warmup operand first so the HAM dummies don't wait
            # for the big vx memset
            wrm = big.tile([128, 512], PV_DT, tag="wrm")
            nc.gpsimd.memset(wrm[:], 0.25)
            # ones columns for v_ext (v evac overwrites the rest)
            nc.gpsimd.memset(vx[:], 1.0)
            # pull the ACT table load off the critical path: a dummy exp
            # right after the memset, long before the first real exp
            nc.scalar.activation(scr[:, 0:4], wrm[0:1, 0:4], EXP)

            # PSUM (16KB/partition, 8 banks):
            #   st  [128,1024]f32 x2 = 4 banks (score tiles, ring of 2)
            #   po  [128, 512]f32 x2 = 2 banks (per-head PV accumulators)
            #   proj[128, 512]f32 x2 = 2 banks (filler projection groups)
            # Boot overlay (8 groups of qk + 16 packed v groups):
            #   stA = q(m0) s0|s1, stB = k(m2) s0|s1,
            #   poA = v01 j0-3, poB = v01 j4-7, proj0 = v01 j8-11,
            #   proj1 = v01 j12-15   (v01 = [128 seq, 128] both heads)
            with tc.tile_pool(name="psum", bufs=2, space="PSUM") as ps, \
                 tc.tile_pool(name="po", bufs=2, space="PSUM") as po:
                bA = ps.tile([128, 1024], F32, tag="st", name="bootA")
                bB = ps.tile([128, 1024], F32, tag="st", name="bootB")
                bP0 = ps.tile([128, 512], F32, tag="proj", name="bootP0")
                bP1 = ps.tile([128, 512], F32, tag="proj", name="bootP1")
                # HAM warmup: junk matmuls over the pre-DMA idle so the
                # boot contractions start at 2.4GHz instead of paying
                # the cold 1.2GHz clock for ~4us.  Round-robin over four
                # banks - back-to-back writes to one bank serialize on
                # the drain.
                warm_t = [bA[:, 0:512], bA[:, 512:1024],
                          bB[:, 0:512], bB[:, 512:1024]]
                for i in range(10):
                    nc.tensor.matmul(warm_t[i % 4], wrm[:, 0:128],
                                     wrm[:], start=True, stop=True)
                vslots = [po.tile([128, 512], F32, tag="po", name="bootVA"),
                          po.tile([128, 512], F32, tag="po", name="bootVB")]
                qk_grp = {
                    (0, 0): bA[:, 0:512], (0, 1): bA[:, 512:1024],
                    (2, 0): bB[:, 0:512], (2, 1): bB[:, 512:1024],
                    (2, 2): bP0[:], (2, 3): bP1[:],
                }

                def vgrp(j):
                    return vslots[j // 4][:, (j % 4) * 128:(j % 4) * 128 + 128]

                def v01_mm(j, ks):
                    # start=True only for the first group in each bank
                    # (start clears has_written bank-wide)
                    for k in ks:
                        nc.tensor.matmul(
                            vgrp(j),
                            xt_t[:, k * D + j * 128:k * D + j * 128 + 128],
                            wv_t[:, k * 256:k * 256 + 128],
                            start=(k == 0 and j % 4 == 0),
                            stop=(k == KT - 1))

                vx4 = vx.rearrange("p (j h x) -> p j h x", h=HPC, x=65)

                def vcopy(j, grp):
                    nc.vector.tensor_copy(
                        vx4[:, j, 0:2, 0:64],
                        grp.rearrange("p (h x) -> p h x", x=64))

                # boot contractions advance with each arriving x chunk;
                # the last chunk is split so the groups gating the first
                # score tiles finish (and evacuate) first
                crit = [(0, 0), (0, 1), (2, 0)]
                for k in range(KT):
                    for (m, s), pt in qk_grp.items():
                        if k == KT - 1 and (m, s) not in crit:
                            continue
                        nc.tensor.matmul(
                            pt, wchunk(m, k),
                            xt_t[:, k * D + s * 512:k * D + (s + 1) * 512],
                            start=(k == 0), stop=(k == KT - 1))
                    for j in range(8):
                        v01_mm(j, [k] if k < KT - 1 else [])
                    if k == KT - 1:
                        v01_mm(0, [k])
                        v01_mm(1, [k])
                # critical evacuations: k s0 on ScalarE, q + first v on
                # DVE - these gate score tile 0 / the first PV steps
                nc.scalar.copy(qkT[:, 2 * D:2 * D + 512], qk_grp[(2, 0)])
                for s in (0, 1):
                    nc.vector.tensor_scalar_add(
                        qkT[:, s * 512:(s + 1) * 512],
                        qk_grp[(0, s)], bq_t[:, 0:1])
                vcopy(0, vgrp(0))
                vcopy(1, vgrp(1))
                # rest of the last chunk, then evacuations on DVE in
                # deadline order (ScalarE stays clear for the exp stream)
                for j in range(2, 8):
                    v01_mm(j, [KT - 1])
                for s in (1, 2, 3):
                    nc.tensor.matmul(
                        qk_grp[(2, s)], wchunk(2, KT - 1),
                        xt_t[:, (KT - 1) * D + s * 512:
                             (KT - 1) * D + (s + 1) * 512],
                        start=False, stop=True)
                vcopy(2, vgrp(2))
                vcopy(3, vgrp(3))
                nc.vector.tensor_copy(qkT[:, 2 * D + 512:2 * D + 1024],
                                      qk_grp[(2, 1)])
                vcopy(4, vgrp(4))
                vcopy(5, vgrp(5))
                nc.vector.tensor_copy(qkT[:, 2 * D + 1024:2 * D + 1536],
                                      qk_grp[(2, 2)])
                vcopy(6, vgrp(6))
                vcopy(7, vgrp(7))
                nc.vector.tensor_copy(qkT[:, 2 * D + 1536:2 * D + 2048],
                                      qk_grp[(2, 3)])

                # ---- fillers: one per attention step, <=~450ns of PE.
                # list order = deadline order; program order guarantees
                # every qkT/vx write precedes its first reader. ----
                fillers = []
                qtiles = {}

                def qk_part(m, s, ks, evac):
                    def emit():
                        key = (m, s)
                        if key not in qtiles:
                            qtiles[key] = ps.tile([128, 512], F32,
                                                  tag="proj",
                                                  name=f"pj{m}{s}")
                        pt = qtiles[key]
                        for k in ks:
                            nc.tensor.matmul(
                                pt[:], wchunk(m, k),
                                xt_t[:, k * D + s * 512:
                                     k * D + (s + 1) * 512],
                                start=(k == 0), stop=(k == KT - 1))
                        if evac:
                            pt = qtiles.pop(key)
                            dst = qkT[:, m * D + s * 512:
                                      m * D + (s + 1) * 512]
                            if m in (0, 1):
                                nc.vector.tensor_scalar_add(
                                    dst, pt[:], bq_t[:, m:m + 1])
                            else:
                                nc.vector.tensor_copy(dst, pt[:])
                    return emit

                def qk_quarters(m, s):
                    return [qk_part(m, s, range(2 * i, 2 * i + 2), i == 3)
                            for i in range(4)]

                vtiles = {}

                def v23_half(j, half):
                    def emit():
                        if half == 0:
                            vtiles[j] = ps.tile([128, 128], F32,
                                                tag="proj", name=f"v23{j}")
                        pt = vtiles[j]
                        for k in (range(4) if half == 0 else range(4, KT)):
                            nc.tensor.matmul(
                                pt[:],
                                xt_t[:, k * D + j * 128:
                                     k * D + j * 128 + 128],
                                wv_t[:, k * 256 + 128:(k + 1) * 256],
                                start=(k == 0), stop=(k == KT - 1))
                        if half == 1:
                            pt = vtiles.pop(j)
                            nc.vector.tensor_copy(
                                vx4[:, j, 2:4, 0:64],
                                pt.rearrange("p (h x) -> p h x", x=64))
                    return emit

                def v01_rest(j):
                    # v heads 0,1 for key tile j (block 0 consumes tile
                    # j at step j; this is popped at step j-8)
                    def emit():
                        pt = ps.tile([128, 128], F32, tag="proj",
                                     name=f"v01r{j}")
                        for k in range(KT):
                            nc.tensor.matmul(
                                pt[:],
                                xt_t[:, k * D + j * 128:
                                     k * D + j * 128 + 128],
                                wv_t[:, k * 256:k * 256 + 128],
                                start=(k == 0), stop=(k == KT - 1))
                        vcopy(j, pt[:])
                    return emit

                # blk0: v01 j8-15, q pair0 qc2/qc3
                for j in range(8, JT):
                    fillers.append(v01_rest(j))
                fillers += qk_quarters(0, 2)
                fillers += qk_quarters(0, 3)
                # blk1: v23 j0-7 first halves+rest
                for j in range(0, 8):
                    fillers += [v23_half(j, 0), v23_half(j, 1)]
                # blk2: v23 j8-13, q pair1 qc0
                for j in range(8, 14):
                    fillers += [v23_half(j, 0), v23_half(j, 1)]
                fillers += qk_quarters(1, 0)
                # blk3: k pair1 s0/s1, q pair1 qc1, v23 j14/j15
                fillers += qk_quarters(3, 0)
                fillers += qk_quarters(3, 1)
                fillers += qk_quarters(1, 1)
                fillers += [v23_half(14, 0), v23_half(14, 1),
                            v23_half(15, 0), v23_half(15, 1)]
                # blk4: k pair1 s2/s3, q pair1 qc2/qc3
                fillers += qk_quarters(3, 2)
                fillers += qk_quarters(3, 3)
                fillers += qk_quarters(1, 2)
                fillers += qk_quarters(1, 3)

                blocks = [
                    _PairBlock(nc, pair, qc, ps, po, es, obp, qkT, vx, o)
                    for pair in range(2) for qc in range(4)
                ]
                blocks[0].emit_qk(0)
                blocks[0].emit_qk(1)
                for i, blk in enumerate(blocks):
                    nxt = blocks[i + 1] if i + 1 < len(blocks) else None
                    blk.emit_loop(fillers, nxt)
                while fillers:
                    fillers.pop(0)()
    nc.compile()
    _nc = nc
    return nc


def _np_dt(dt):
    if dt == BF16:
        return ml_dtypes.bfloat16
    if dt == F16:
        return np.float16
    return np.float32


def _shard_inputs(x, W_qkv, b_qkv):
    pj = _np_dt(PJ_DT)
    in_maps = []
    for c in range(NCORES):
        b = c // 4
        h0 = HPC * (c % 4)
        xT = np.ascontiguousarray(x[b].T).astype(pj)
        wq = W_qkv[:, h0 * P:(h0 + HPC) * P]
        wk = W_qkv[:, N + h0 * P:N + (h0 + HPC) * P]
        wqk_cols = np.concatenate([wq, wk], axis=1)  # [N, 512]
        wqk = np.ascontiguousarray(
            wqk_cols.reshape(N, 4, 128).transpose(1, 0, 2)
            .reshape(4 * N, 128)).astype(pj)
        wv = np.ascontiguousarray(
            W_qkv[:, 2 * N + h0 * P:2 * N + (h0 + HPC) * P]).astype(pj)
        bqh = b_qkv[h0 * P:(h0 + HPC) * P]           # q bias only
        bqc = np.ascontiguousarray(
            bqh.reshape(2, 128).T).astype(np.float32)
        in_maps.append({"xt": xT, "wqk": wqk, "wv": wv, "bq": bqc})
    return in_maps


def _assemble(results, b_qkv):
    out = np.empty((B, D, N), dtype=np.float32)
    for c in range(NCORES):
        b = c // 4
        h0 = HPC * (c % 4)
        oe = results[c]["o"]                      # (4, 65, 2048)
        # even heads: v rows 0-63, denom row 64; odd: denom 0, v 1-64
        att = np.empty((HPC, P, D), dtype=np.float32)
        att[0::2] = oe[0::2, :P, :] / oe[0::2, P:P + 1, :]
        att[1::2] = oe[1::2, 1:P + 1, :] / oe[1::2, 0:1, :]
        att = np.transpose(att, (0, 2, 1))        # (4, 2048, 64)
        for hl in range(HPC):
            h = h0 + hl
            bv = b_qkv[2 * N + h * P:2 * N + (h + 1) * P]
            out[b, h * 128:(h + 1) * 128, :] = \
                (att[hl] + bv[None, :]).reshape(128, N)
    return out


def _forward(in_maps, **kwargs):
    nc = _build()
    return run_bass_kernel_spmd(nc, in_maps, core_ids=list(range(NCORES)),
                                **kwargs)


def kernel(x, W_qkv, b_qkv):
    x = np.asarray(x, dtype=np.float32)
    W_qkv = np.asarray(W_qkv, dtype=np.float32)
    b_qkv = np.asarray(b_qkv, dtype=np.float32)
    in_maps = _shard_inputs(x, W_qkv, b_qkv)
    res = _forward(in_maps)
    return _assemble(res.results, b_qkv)


# revision 25
# speedup vs baseline: 1.0082x; 1.0082x over previous
"""Multi-head attention (B=2, D=2048, N=1024, H=16) on 8 TRN2 NeuronCores.

Sharding: batch*heads across cores - core c handles batch c//4, heads
4*(c%4) .. 4*(c%4)+3. No collectives.

The attention phase is ScalarE-bound: exp of 16.8M score elements per
core at 1 elem/cycle/lane @ 1.2GHz is ~143us of ACTIVATE, which the PE
cannot outrun.  The kernel is organized to (a) start the exp stream as
early as possible and (b) hide ALL projection work in PE idle during the
ACT-bound attention phase:

  1. Inputs in fp16 (halves the x DMA; q/k are consumed at fp16 and v at
     bf16 downstream anyway).
  2. Head-PAIR packed blocks: the pair's two heads occupy partitions
     0-63 / 64-127 of the q/k buffer, so their score matmuls run
     CONCURRENTLY as 64x128 row-tiles writing the two bank-halves of one
     [128,1024] PSUM tile; one ACTIVATE exps both heads; PV accumulates
     into per-head [65,512] tiles (ones column = softmax denominator).
     Per-step PE work (~640ns) is well under the 1114ns ACT period.
  3. Boot: every PSUM bank holds a projection accumulator so the
     contractions advance chunk-by-chunk as x streams in: q(pair0,s0|s1)
     + k(pair0,j0-7) + v(heads01, all 16 key tiles, packed [128,128])
     = exactly 8 banks.  PSUM rule: a matmul's start=True clears
     has_written for its whole BANK, so only the first group per bank
     uses start=True; later groups in the bank rely on
     overwrite-where-clear.  The k bias cancels in softmax (j-constant
     shift), so k evacuations are plain ScalarE copies; q bias-adds and
     v copies run on DVE.
  4. Remaining projections are quarter/half-size filler matmul groups
     (<=~450ns) woven one per attention step, ordered so every write
     precedes its first reader in program order with slack.  Each
     block's first two score tiles are emitted in the previous block's
     tail steps so the ACTIVATE stream never pauses at block boundaries.

No softmax max-subtraction: scores are ~N(0, 8^2), exp fits fp32.
Host post-pass: divide by denominator, add the (linearly separable) v
bias, transpose + reshape into the reference's raw (B,H,D,p)->(B,D,N).
"""
import sys

sys.path.insert(0, "/opt/trn_rl_repo")

import numpy as np
import ml_dtypes
import concourse.bacc as bacc
import concourse.mybir as mybir
from concourse import tile
from concourse.bass_utils import run_bass_kernel_spmd

B, D, N, H, P = 2, 2048, 1024, 16, 64
NCORES = 8
HPC = 4            # heads per core
KT = 8             # contraction tiles (N / 128)
JT = 16            # j (key) tiles of 128 per head
F32 = mybir.dt.float32
BF16 = mybir.dt.bfloat16
F16 = mybir.dt.float16
EXP = mybir.ActivationFunctionType.Exp

PJ_DT = F16        # projection operands (x, W)
QK_DT = F16        # q/k tiles feeding the scores matmul
PV_DT = BF16       # expS + v_ext feeding the PV matmul

_nc = None


class _PairBlock:
    """One (pair, qc) flash-attention block: heads 2*pair, 2*pair+1,
    query chunk qc (512 wide), 16 flash steps over key tiles."""

    def __init__(self, nc, pair, qc, ps, po, es, obp, qkT, vx, o):
        self.nc, self.pair, self.qc = nc, pair, qc
        self.ps, self.po, self.es, self.obp = ps, po, es, obp
        self.qkT, self.vx, self.o = qkT, vx, o
        self.qoff = pair * D + qc * 512
        self.koff = (2 + pair) * D
        self.sts = {}

    def emit_qk(self, j):
        # both heads concurrently: head A on partitions 0-63 (row tile
        # (0,0)) -> bank half 0, head B on 64-127 ((64,0)) -> half 1
        nc, qkT = self.nc, self.qkT
        st = self.ps.tile([128, 1024], F32, tag="st",
                          name=f"st{self.pair}{self.qc}")
        for hb in range(2):
            nc.tensor.matmul(
                st[:, hb * 512:(hb + 1) * 512],
                qkT[hb * 64:hb * 64 + 64,
                    self.koff + j * 128:self.koff + (j + 1) * 128],
                qkT[hb * 64:hb * 64 + 64, self.qoff:self.qoff + 512],
                start=True, stop=True)
        self.sts[j] = st

    def emit_loop(self, fillers, nxt):
        nc = self.nc
        ots = [self.po.tile([P + 1, 512], F32, tag="po",
                            name=f"ot{self.pair}{self.qc}{hb}")
               for hb in range(2)]
        for j in range(JT):
            et = self.es.tile([128, 1024], PV_DT, tag="et",
                              name=f"et{self.pair}{self.qc}")
            nc.scalar.activation(et[:], self.sts.pop(j)[:], EXP)
            if j + 2 < JT:
                self.emit_qk(j + 2)
            elif nxt is not None:
                nxt.emit_qk(j - 14)
            for hb in range(2):
                # even head: [v|ones] -> denominator in row 64;
                # odd head: [ones|v] (shared ones col) -> denom in row 0
                h = self.pair * 2 + hb
                off = j * 260 + h * 65 - (h % 2)
                nc.tensor.matmul(
                    ots[hb][:],
                    self.vx[:, off:off + 65],
                    et[:, hb * 512:(hb + 1) * 512],
                    start=(j == 0), stop=(j == JT - 1))
            if fillers:
                fillers.pop(0)()
        ob = self.obp.tile([P + 1, 1024], F32, tag="ob")
        orr = self.o.rearrange("h p d -> (h p) d")
        for hb in range(2):
            h = self.pair * 2 + hb
            if nxt is None and hb == 1:
                # last block: ScalarE is idle after the final exp -
                # evacuate the two heads on different engines in parallel
                self.nc.scalar.copy(ob[:, 512:1024], ots[1][:])
            else:
                nc.vector.tensor_copy(ob[:, hb * 512:(hb + 1) * 512],
                                      ots[hb][:])
            nc.sync.dma_start(
                out=orr[h * 65:(h + 1) * 65,
                        self.qc * 512:(self.qc + 1) * 512],
                in_=ob[:, hb * 512:(hb + 1) * 512])


def _build():
    global _nc
    if _nc is not None:
        return _nc
    nc = bacc.Bacc("TRN2", target_bir_lowering=False, debug=False,
                   num_devices=NCORES)
    xt = nc.dram_tensor("xt", [N, D], PJ_DT, kind="ExternalInput").ap()
    # wqk m-major: m in {q lo, q hi, k lo, k hi}, rows m*N + n
    wqk = nc.dram_tensor("wqk", [4 * N, 128], PJ_DT,
                         kind="ExternalInput").ap()
    # boot weights, contiguous per channel row: [q lo | k lo | wv] so one
    # fast DMA descriptor (1KB runs) lands them before the first x chunk
    wb = nc.dram_tensor("wb", [N, 512], PJ_DT, kind="ExternalInput").ap()
    bq = nc.dram_tensor("bq", [128, 2], F32, kind="ExternalInput").ap()
    o = nc.dram_tensor("o", [HPC, P + 1, D], F32, kind="ExternalOutput").ap()

    with tile.TileContext(nc) as tc:
        with tc.tile_pool(name="big", bufs=1) as big, \
             tc.tile_pool(name="es", bufs=10) as es, \
             tc.tile_pool(name="obp", bufs=2) as obp:

            xt_t = big.tile([128, KT * D], PJ_DT, tag="xt")
            wqk_t = big.tile([128, 4 * KT * 128], PJ_DT, tag="wqk")
            wb_t = big.tile([128, KT * 512], PJ_DT, tag="wb")
            bq_t = big.tile([128, 2], F32, tag="bq")
            qkT = big.tile([128, 4 * D], QK_DT, tag="qkT")
            vx = big.tile([128, JT * HPC * 65], PV_DT, tag="vx")
            scr = big.tile([1, 8], F32, tag="scr")

            def wchunk(m, k):
                if m in (0, 2):   # boot weights live chunk-major in wb_t
                    c = k * 512 + (m // 2) * 128
                else:
                    c = m * 1024 + k * 128
                    return wqk_t[:, c:c + 128]
                return wb_t[:, c:c + 128]

            def wvchunk(k, lo, hi):
                return wb_t[:, k * 512 + 256 + lo:k * 512 + 256 + hi]

            def dma_w(m):
                nc.sync.dma_start(
                    out=wqk_t[:, m * 1024:(m + 1) * 1024].rearrange(
                        "p (k c) -> p k c", c=128),
                    in_=wqk[m * N:(m + 1) * N, :].rearrange(
                        "(k p) c -> p k c", p=128))

            # DMA instructions issue serially at ~0.8us each on the Sync
            # queue and the boot is gated on the weights: one contiguous
            # descriptor for all boot weights, then the 8 x chunks
            nc.sync.dma_start(
                out=wb_t[:].rearrange("p (k c) -> p k c", c=512),
                in_=wb.rearrange("(k p) c -> p k c", p=128))
            for k in range(KT):
                nc.sync.dma_start(out=xt_t[:, k * D:(k + 1) * D],
                                  in_=xt[k * 128:(k + 1) * 128, :])
            nc.sync.dma_start(out=bq_t[:], in_=bq)
            for m in (1, 3):
                dma_w(m)
            # small warmup operand first so the HAM dummies don't wait
            # for the big vx memset
            wrm = big.tile([128, 512], PV_DT, tag="wrm")
            nc.gpsimd.memset(wrm[:], 0.25)
            # ones columns for v_ext (v evac overwrites the rest)
            nc.gpsimd.memset(vx[:], 1.0)
            # pull the ACT table load off the critical path: a dummy exp
            # right after the memset, long before the first real exp
            nc.scalar.activation(scr[:, 0:4], wrm[0:1, 0:4], EXP)

            # PSUM (16KB/partition, 8 banks):
            #   st  [128,1024]f32 x2 = 4 banks (score tiles, ring of 2)
            #   po  [128, 512]f32 x2 = 2 banks (per-head PV accumulators)
            #   proj[128, 512]f32 x2 = 2 banks (filler projection groups)
            # Boot overlay (8 groups of qk + 16 packed v groups):
            #   stA = q(m0) s0|s1, stB = k(m2) s0|s1,
            #   poA = v01 j0-3, poB = v01 j4-7, proj0 = v01 j8-11,
            #   proj1 = v01 j12-15   (v01 = [128 seq, 128] both heads)
            with tc.tile_pool(name="psum", bufs=2, space="PSUM") as ps, \
                 tc.tile_pool(name="po", bufs=2, space="PSUM") as po:
                bA = ps.tile([128, 1024], F32, tag="st", name="bootA")
                bB = ps.tile([128, 1024], F32, tag="st", name="bootB")
                bP0 = ps.tile([128, 512], F32, tag="proj", name="bootP0")
                bP1 = ps.tile([128, 512], F32, tag="proj", name="bootP1")
                # HAM warmup: junk matmuls over the pre-DMA idle so the
                # boot contractions start at 2.4GHz instead of paying
                # the cold 1.2GHz clock for ~4us.  Round-robin over four
                # banks - back-to-back writes to one bank serialize on
                # the drain.
                warm_t = [bA[:, 0:512], bA[:, 512:1024],
                          bB[:, 0:512], bB[:, 512:1024]]
                for i in range(10):
                    nc.tensor.matmul(warm_t[i % 4], wrm[:, 0:128],
                                     wrm[:], start=True, stop=True)
                vslots = [po.tile([128, 512], F32, tag="po", name="bootVA"),
                          po.tile([128, 512], F32, tag="po", name="bootVB")]
                qk_grp = {
                    (0, 0): bA[:, 0:512], (0, 1): bA[:, 512:1024],
                    (2, 0): bB[:, 0:512], (2, 1): bB[:, 512:1024],
                    (2, 2): bP0[:], (2, 3): bP1[:],
                }

                def vgrp(j):
                    return vslots[j // 4][:, (j % 4) * 128:(j % 4) * 128 + 128]

                def v01_mm(j, ks):
                    # start=True only for the first group in each bank
                    # (start clears has_written bank-wide)
                    for k in ks:
                        nc.tensor.matmul(
                            vgrp(j),
                            xt_t[:, k * D + j * 128:k * D + j * 128 + 128],
                            wvchunk(k, 0, 128),
                            start=(k == 0 and j % 4 == 0),
                            stop=(k == KT - 1))

                vx4 = vx.rearrange("p (j h x) -> p j h x", h=HPC, x=65)

                def vcopy(j, grp):
                    nc.vector.tensor_copy(
                        vx4[:, j, 0:2, 0:64],
                        grp.rearrange("p (h x) -> p h x", x=64))

                # boot contractions advance with each arriving x chunk;
                # the last chunk is split so the groups gating the first
                # score tiles finish (and evacuate) first
                crit = [(0, 0), (0, 1), (2, 0)]
                for k in range(KT):
                    for (m, s), pt in qk_grp.items():
                        if k == KT - 1 and (m, s) not in crit:
                            continue
                        nc.tensor.matmul(
                            pt, wchunk(m, k),
                            xt_t[:, k * D + s * 512:k * D + (s + 1) * 512],
                            start=(k == 0), stop=(k == KT - 1))
                    for j in range(8):
                        v01_mm(j, [k] if k < KT - 1 else [])
                    if k == KT - 1:
                        v01_mm(0, [k])
                        v01_mm(1, [k])
                # critical evacuations: k s0 on ScalarE, q + first v on
                # DVE - these gate score tile 0 / the first PV steps
                nc.scalar.copy(qkT[:, 2 * D:2 * D + 512], qk_grp[(2, 0)])
                for s in (0, 1):
                    nc.vector.tensor_scalar_add(
                        qkT[:, s * 512:(s + 1) * 512],
                        qk_grp[(0, s)], bq_t[:, 0:1])
                vcopy(0, vgrp(0))
                vcopy(1, vgrp(1))
                # rest of the last chunk, then evacuations on DVE in
                # deadline order (ScalarE stays clear for the exp stream)
                for j in range(2, 8):
                    v01_mm(j, [KT - 1])
                for s in (1, 2, 3):
                    nc.tensor.matmul(
                        qk_grp[(2, s)], wchunk(2, KT - 1),
                        xt_t[:, (KT - 1) * D + s * 512:
                             (KT - 1) * D + (s + 1) * 512],
                        start=False, stop=True)
                vcopy(2, vgrp(2))
                vcopy(3, vgrp(3))
                nc.vector.tensor_copy(qkT[:, 2 * D + 512:2 * D + 1024],
                                      qk_grp[(2, 1)])
                vcopy(4, vgrp(4))
                vcopy(5, vgrp(5))
                nc.vector.tensor_copy(qkT[:, 2 * D + 1024:2 * D + 1536],
                                      qk_grp[(2, 2)])
                vcopy(6, vgrp(6))
                vcopy(7, vgrp(7))
                nc.vector.tensor_copy(qkT[:, 2 * D + 1536:2 * D + 2048],
                                      qk_grp[(2, 3)])

                # ---- fillers: one per attention step, <=~450ns of PE.
                # list order = deadline order; program order guarantees
                # every qkT/vx write precedes its first reader. ----
                fillers = []
                qtiles = {}

                def qk_part(m, s, ks, evac):
                    def emit():
                        key = (m, s)
                        if key not in qtiles:
                            qtiles[key] = ps.tile([128, 512], F32,
                                                  tag="proj",
                                                  name=f"pj{m}{s}")
                        pt = qtiles[key]
                        for k in ks:
                            nc.tensor.matmul(
                                pt[:], wchunk(m, k),
                                xt_t[:, k * D + s * 512:
                                     k * D + (s + 1) * 512],
                                start=(k == 0), stop=(k == KT - 1))
                        if evac:
                            pt = qtiles.pop(key)
                            dst = qkT[:, m * D + s * 512:
                                      m * D + (s + 1) * 512]
                            if m in (0, 1):
                                nc.vector.tensor_scalar_add(
                                    dst, pt[:], bq_t[:, m:m + 1])
                            else:
                                nc.vector.tensor_copy(dst, pt[:])
                    return emit

                def qk_quarters(m, s):
                    return [qk_part(m, s, range(2 * i, 2 * i + 2), i == 3)
                            for i in range(4)]

                vtiles = {}

                def v23_half(j, half):
                    def emit():
                        if half == 0:
                            vtiles[j] = ps.tile([128, 128], F32,
                                                tag="proj", name=f"v23{j}")
                        pt = vtiles[j]
                        for k in (range(4) if half == 0 else range(4, KT)):
                            nc.tensor.matmul(
                                pt[:],
                                xt_t[:, k * D + j * 128:
                                     k * D + j * 128 + 128],
                                wvchunk(k, 128, 256),
                                start=(k == 0), stop=(k == KT - 1))
                        if half == 1:
                            pt = vtiles.pop(j)
                            nc.vector.tensor_copy(
                                vx4[:, j, 2:4, 0:64],
                                pt.rearrange("p (h x) -> p h x", x=64))
                    return emit

                def v01_rest(j):
                    # v heads 0,1 for key tile j (block 0 consumes tile
                    # j at step j; this is popped at step j-8)
                    def emit():
                        pt = ps.tile([128, 128], F32, tag="proj",
                                     name=f"v01r{j}")
                        for k in range(KT):
                            nc.tensor.matmul(
                                pt[:],
                                xt_t[:, k * D + j * 128:
                                     k * D + j * 128 + 128],
                                wvchunk(k, 0, 128),
                                start=(k == 0), stop=(k == KT - 1))
                        vcopy(j, pt[:])
                    return emit

                # blk0: v01 j8-15, q pair0 qc2/qc3
                for j in range(8, JT):
                    fillers.append(v01_rest(j))
                fillers += qk_quarters(0, 2)
                fillers += qk_quarters(0, 3)
                # blk1: v23 j0-7 first halves+rest
                for j in range(0, 8):
                    fillers += [v23_half(j, 0), v23_half(j, 1)]
                # blk2: v23 j8-13, q pair1 qc0
                for j in range(8, 14):
                    fillers += [v23_half(j, 0), v23_half(j, 1)]
                fillers += qk_quarters(1, 0)
                # blk3: k pair1 s0/s1, q pair1 qc1, v23 j14/j15
                fillers += qk_quarters(3, 0)
                fillers += qk_quarters(3, 1)
                fillers += qk_quarters(1, 1)
                fillers += [v23_half(14, 0), v23_half(14, 1),
                            v23_half(15, 0), v23_half(15, 1)]
                # blk4: k pair1 s2/s3, q pair1 qc2/qc3
                fillers += qk_quarters(3, 2)
                fillers += qk_quarters(3, 3)
                fillers += qk_quarters(1, 2)
                fillers += qk_quarters(1, 3)

                blocks = [
                    _PairBlock(nc, pair, qc, ps, po, es, obp, qkT, vx, o)
                    for pair in range(2) for qc in range(4)
                ]
                blocks[0].emit_qk(0)
                blocks[0].emit_qk(1)
                for i, blk in enumerate(blocks):
                    nxt = blocks[i + 1] if i + 1 < len(blocks) else None
                    blk.emit_loop(fillers, nxt)
                while fillers:
                    fillers.pop(0)()
    nc.compile()
    _nc = nc
    return nc


def _np_dt(dt):
    if dt == BF16:
        return ml_dtypes.bfloat16
    if dt == F16:
        return np.float16
    return np.float32


def _shard_inputs(x, W_qkv, b_qkv):
    pj = _np_dt(PJ_DT)
    in_maps = []
    for c in range(NCORES):
        b = c // 4
        h0 = HPC * (c % 4)
        xT = np.ascontiguousarray(x[b].T).astype(pj)
        wq = W_qkv[:, h0 * P:(h0 + HPC) * P]
        wk = W_qkv[:, N + h0 * P:N + (h0 + HPC) * P]
        wqk_cols = np.concatenate([wq, wk], axis=1)  # [N, 512]
        wqk = np.ascontiguousarray(
            wqk_cols.reshape(N, 4, 128).transpose(1, 0, 2)
            .reshape(4 * N, 128)).astype(pj)
        wv_cols = W_qkv[:, 2 * N + h0 * P:2 * N + (h0 + HPC) * P]
        wb = np.ascontiguousarray(np.concatenate(
            [wqk_cols[:, 0:128], wqk_cols[:, 256:384], wv_cols],
            axis=1)).astype(pj)
        bqh = b_qkv[h0 * P:(h0 + HPC) * P]           # q bias only
        bqc = np.ascontiguousarray(
            bqh.reshape(2, 128).T).astype(np.float32)
        in_maps.append({"xt": xT, "wqk": wqk, "wb": wb, "bq": bqc})
    return in_maps


def _assemble(results, b_qkv):
    out = np.empty((B, D, N), dtype=np.float32)
    for c in range(NCORES):
        b = c // 4
        h0 = HPC * (c % 4)
        oe = results[c]["o"]                      # (4, 65, 2048)
        # even heads: v rows 0-63, denom row 64; odd: denom 0, v 1-64
        att = np.empty((HPC, P, D), dtype=np.float32)
        att[0::2] = oe[0::2, :P, :] / oe[0::2, P:P + 1, :]
        att[1::2] = oe[1::2, 1:P + 1, :] / oe[1::2, 0:1, :]
        att = np.transpose(att, (0, 2, 1))        # (4, 2048, 64)
        for hl in range(HPC):
            h = h0 + hl
            bv = b_qkv[2 * N + h * P:2 * N + (h + 1) * P]
            out[b, h * 128:(h + 1) * 128, :] = \
                (att[hl] + bv[None, :]).reshape(128, N)
    return out


def _forward(in_maps, **kwargs):
    nc = _build()
    return run_bass_kernel_spmd(nc, in_maps, core_ids=list(range(NCORES)),
                                **kwargs)


def kernel(x, W_qkv, b_qkv):
    x = np.asarray(x, dtype=np.float32)
    W_qkv = np.asarray(W_qkv, dtype=np.float32)
    b_qkv = np.asarray(b_qkv, dtype=np.float32)
    in_maps = _shard_inputs(x, W_qkv, b_qkv)
    res = _forward(in_maps)
    return _assemble(res.results, b_qkv)
